# revision 41
# baseline (speedup 1.0000x reference)
"""Trainium2 Bass kernel for FlashMultiHeadAttention (B=2, L=2048, D=1024, H=16, Dh=64).

Sharding: 8 cores = 2 (batch) x 4 (head groups of 4 heads).
Per core (batch b, head group hg, 4 heads):
  - Q^T/K^T projections ([256, L], head dims on partitions, 8 k-tiles; bq/bk
    folded into the ACT-engine PSUM->SBUF evacuation as per-partition biases).
    RoPE applied by the DVE entirely in SBUF bf16 (4x DVE rate) on the ACT
    copies.  U (9 k-tiles, action-gate + biases via augmented rows) and V
    (9 k-tiles, ones-row bias) projected in natural [L, 256] layout; sigmoid
    via ACT tanh; gating mul fused into one DVE op per chunk.
  - Scores computed transposed (S^T[k, q]); exp producers alternate by key
    block between the ACT engine (exp with fused scale+bias+cast) and the DVE
    (Schraudolph int16 bitcast-to-bf16 approximation) so softmax never paces
    the PE.  P^T feeds PV directly; PV carries a denominator ones-column.
  - Normalization: fast-approx reciprocal + DRAM-broadcast of 1/r; the final
    per-head scale runs on GpSimd (SBUF only), off every critical engine.
  - Row-sliced output projection -> partial outT [1024, 2048] bf16, with the
    two 128-row halves of the contraction accumulated n2-major so the last
    head's normalize overlaps the first half of the out-projection.
Host sums the 4 head-group partials per batch and adds bo.

One PSUM pool with two 2-bank tags ("qp"/"up", bufs=2 each = 8 banks) is
shared by every phase so there are no pool-release barriers.  Input DMAs are
spread over four rings (SP: xq+xv, Pool: xk+wk/wv/wo, PE: wq/wu, ACT: small
tables + first xq chunk tail) so no single ring serializes the head.
"""

import sys

if "/opt/trn_rl_repo" not in sys.path:
    sys.path.insert(0, "/opt/trn_rl_repo")

import numpy as np
import ml_dtypes

BF16 = ml_dtypes.bfloat16

B = 2
L = 2048
D = 1024
H = 16
DH = 64
NG = 256          # head dims per group (4 heads)
NCORES = 8
SCALE = DH ** -0.5

# Schraudolph exp approximation constants (bf16 bit layout via int16).
A16 = 128.0 / float(np.log(2.0))
SIGMA = 0.0579
B0 = 128.0 * (127.0 - SIGMA)
# exp producer split: these key blocks' exp runs on the DVE (Schraudolph),
# the rest on the ACT engine, so the two producers run concurrently and
# neither paces the PE.
DVE_KT = (2, 5, 8, 10, 13, 15)


def build_bass(with_mask: bool):
    """Build the single-core SPMD Bass program (same program on all 8 cores)."""
    import concourse.mybir as mybir
    from concourse import bacc
    from concourse.tile import TileContext

    f32 = mybir.dt.float32
    bf16 = mybir.dt.bfloat16
    i16 = mybir.dt.int16
    EXP = mybir.ActivationFunctionType.Exp
    TANH = mybir.ActivationFunctionType.Tanh
    IDENT = mybir.ActivationFunctionType.Identity
    COPY = mybir.ActivationFunctionType.Copy
    MULT = mybir.AluOpType.mult
    ADD = mybir.AluOpType.add

    def use_dve(kt):
        return (not with_mask) and kt in DVE_KT

    nc = bacc.Bacc(None, target_bir_lowering=False)

    xq = nc.dram_tensor("xq", [9 * 128, L], bf16, kind="ExternalInput")
    xk = nc.dram_tensor("xk", [8 * 128, L], bf16, kind="ExternalInput")
    xv = nc.dram_tensor("xv", [9 * 128, L], bf16, kind="ExternalInput")
    wq = nc.dram_tensor("wq", [8 * 128, NG], bf16, kind="ExternalInput")
    wu = nc.dram_tensor("wu", [9 * 128, NG], bf16, kind="ExternalInput")
    wk = nc.dram_tensor("wk", [8 * 128, NG], bf16, kind="ExternalInput")
    wv = nc.dram_tensor("wv", [9 * 128, NG], bf16, kind="ExternalInput")
    wo = nc.dram_tensor("wo", [NG, D], bf16, kind="ExternalInput")
    cb = nc.dram_tensor("cb", [128, 64], f32, kind="ExternalInput")
    bqk = nc.dram_tensor("bqk", [128, 4], f32, kind="ExternalInput")
    cs = nc.dram_tensor("cs", [128, L], bf16, kind="ExternalInput")
    sn = nc.dram_tensor("sn", [128, L], bf16, kind="ExternalInput")
    mk = None
    if with_mask:
        mk = nc.dram_tensor("mk", [L, L], f32, kind="ExternalInput")
    outT = nc.dram_tensor("outT", [D, L], bf16, kind="ExternalOutput")

    with TileContext(nc) as tc:
        with tc.tile_pool(name="persist", bufs=1) as persist, \
             tc.tile_pool(name="xbuf", bufs=1) as xbuf, \
             tc.tile_pool(name="ps", bufs=1, space="PSUM") as ps, \
             tc.tile_pool(name="ev", bufs=2) as ev, \
             tc.tile_pool(name="ptp", bufs=8) as ptpool, \
             tc.tile_pool(name="otp", bufs=2) as otpool, \
             tc.tile_pool(name="drm", bufs=2, space="DRAM") as drm, \
             tc.tile_pool(name="mkp", bufs=4) as mkpool:
            qT = [persist.tile([128, L], bf16, name=f"qT{n}") for n in range(2)]
            kT = [persist.tile([128, L], bf16, name=f"kT{n}") for n in range(2)]
            vg = persist.tile([128, 16 * 260], bf16, name="vg")
            vg4 = vg.rearrange("p (t h e) -> p t h e", h=4, e=65)
            sig = [persist.tile([128, 1024], bf16, name=f"sig{c}") for c in range(4)]
            oT = [persist.tile([128, L], bf16, name=f"oT{n}") for n in range(2)]
            csS = persist.tile([128, L], bf16, name="csS")
            snS = persist.tile([128, L], bf16, name="snS")
            cbS = persist.tile([128, 64], f32, name="cbS")
            cbA = persist.tile([128, 64], f32, name="cbA")
            bqkS = persist.tile([128, 4], f32, name="bqkS")
            woS = [persist.tile([128, D], bf16, name=f"woS{n2}") for n2 in range(2)]
            wqS = persist.tile([128, 8 * NG], bf16, name="wqS")
            wuS = persist.tile([128, 9 * NG], bf16, name="wuS")
            wkS = persist.tile([128, 8 * NG], bf16, name="wkS")
            wvS = persist.tile([128, 9 * NG], bf16, name="wvS")
            ib = persist.tile([128, L], f32, name="ib")
            rg = persist.tile([128, 512], f32, name="rg")
            rinv = persist.tile([128, 512], f32, name="rinv")
            rinvb = [persist.tile([128, 512], bf16, name=f"rinvb{j}")
                     for j in range(2)]

            wqS3 = wqS.rearrange("p (c n) -> p c n", n=NG)
            wuS3 = wuS.rearrange("p (c n) -> p c n", n=NG)
            wkS3 = wkS.rearrange("p (c n) -> p c n", n=NG)
            wvS3 = wvS.rearrange("p (c n) -> p c n", n=NG)

            xqS = xbuf.tile([128, 9 * L], bf16, tag="xA", name="xqS")
            xqS3 = xqS.rearrange("p (c q) -> p c q", q=L)
            xkS = xbuf.tile([128, 8 * L], bf16, tag="xB", name="xkS")
            xkS3 = xkS.rearrange("p (c q) -> p c q", q=L)

            # ---- input DMAs spread over three rings ----
            # Pool ring: xq chunk 1 (the SP ring alone can't keep ahead of
            # the PE), then rope tables, xk, K/V weights, wo.
            for d in range(9):
                nc.gpsimd.dma_start(out=xqS3[:, d, 512:1024],
                                    in_=xq[d * 128:(d + 1) * 128, 512:1024])
            nc.gpsimd.dma_start(out=csS, in_=cs[:, :])
            nc.gpsimd.dma_start(out=snS, in_=sn[:, :])
            for d in range(8):
                nc.gpsimd.dma_start(out=xkS3[:, d, 0:512],
                                    in_=xk[d * 128:(d + 1) * 128, 0:512])
            nc.gpsimd.dma_start(out=wkS.rearrange("p (c n) -> p c n", n=NG),
                                in_=wk.rearrange("(c p) n -> p c n", p=128))
            nc.gpsimd.dma_start(out=wvS.rearrange("p (c n) -> p c n", n=NG),
                                in_=wv.rearrange("(c p) n -> p c n", p=128))
            for c in range(1, 4):
                s = slice(c * 512, (c + 1) * 512)
                for d in range(8):
                    nc.gpsimd.dma_start(out=xkS3[:, d, s],
                                        in_=xk[d * 128:(d + 1) * 128, s])
            for n2 in range(2):
                nc.gpsimd.dma_start(out=woS[n2], in_=wo[n2 * 128:(n2 + 1) * 128, :])
            # ACT ring: wq/wu ahead of the first matmuls, then small tables.
            nc.scalar.dma_start(out=wqS3, in_=wq.rearrange("(c p) n -> p c n", p=128))
            nc.scalar.dma_start(out=bqkS, in_=bqk[:, :])
            nc.scalar.dma_start(out=wuS3, in_=wu.rearrange("(c p) n -> p c n", p=128))
            nc.scalar.dma_start(out=cbS, in_=cb[:, :])
            # SP ring: xq chunks 0, 2, 3.
            for c in (0, 2, 3):
                s = slice(c * 512, (c + 1) * 512)
                for d in range(9):
                    nc.sync.dma_start(out=xqS3[:, d, s],
                                      in_=xq[d * 128:(d + 1) * 128, s])

            # device-side preamble computations
            nc.vector.tensor_scalar(cbA, cbS, A16, B0, MULT, ADD)
            nc.vector.memset(vg4[:, :, :, 64:65], 1.0)
            nc.gpsimd.memset(rg, 1.0)

            # head dims are stored pair-interleaved (partner of p is p^1), so
            # rotate_half is a swap of adjacent partitions within quadrants.
            SWAP_MASK = [i ^ 1 for i in range(32)]

            def rope_sbuf(raw, dest, s):
                """dest[:, s] = raw*cos + rotate_half(raw)*signed_sin (bf16 SBUF)."""
                for n in range(2):
                    src = raw[:, n * 512:(n + 1) * 512]
                    tcx = ev.tile([128, 512], bf16, tag="tc", bufs=2, name="tcx")
                    rot = ev.tile([128, 512], bf16, tag="tr", bufs=2, name="rot")
                    nc.vector.tensor_mul(tcx, src, csS[:, s])
                    nc.vector.stream_shuffle(rot, src, SWAP_MASK)
                    nc.vector.tensor_mul(rot, rot, snS[:, s])
                    nc.vector.tensor_add(dest[n][:, s], tcx, rot)

            # ---- QU phase ----
            for c in range(4):
                s = slice(c * 512, (c + 1) * 512)
                qps = ps.tile([128, 1024], f32, tag="qp", bufs=3, name="qps")
                for d in range(8):
                    xt = xqS3[:, d, s]
                    for n in range(2):
                        nc.tensor.matmul(qps[:, n * 512:(n + 1) * 512],
                                         lhsT=wqS3[:, d, n * 128:(n + 1) * 128],
                                         rhs=xt, start=(d == 0), stop=(d == 7))
                qraw = ev.tile([128, 1024], bf16, tag="qraw", bufs=2, name="qraw")
                for n in range(2):
                    nc.scalar.activation(out=qraw[:, n * 512:(n + 1) * 512],
                                         in_=qps[:, n * 512:(n + 1) * 512],
                                         func=IDENT, bias=bqkS[:, n:n + 1])
                rope_sbuf(qraw, qT, s)
                ups = ps.tile([128, 1024], f32, tag="up", bufs=1, name="ups")
                for i in range(4):
                    for d in range(9):
                        nc.tensor.matmul(ups[:, i * 256:(i + 1) * 256],
                                         lhsT=xqS3[:, d, s][:, i * 128:(i + 1) * 128],
                                         rhs=wuS3[:, d, :],
                                         start=(d == 0), stop=(d == 8))
                eu = ev.tile([128, 1024], bf16, tag="eu", bufs=2, name="eu")
                nc.scalar.activation(out=eu, in_=ups, func=TANH, scale=0.5)
                # sigmoid(u) = 0.5*tanh(0.5*u) + 0.5
                nc.vector.tensor_scalar(sig[c], eu, 0.5, 0.5, MULT, ADD)

            # ---- KV phase (xv reuses the xq SBUF bytes) ----
            xvS = xbuf.tile([128, 9 * L], bf16, tag="xA", name="xvS")
            xvS3 = xvS.rearrange("p (c q) -> p c q", q=L)
            for c in range(4):
                s = slice(c * 512, (c + 1) * 512)
                for d in range(9):
                    nc.sync.dma_start(out=xvS3[:, d, s],
                                      in_=xv[d * 128:(d + 1) * 128, s])
            for c in range(4):
                s = slice(c * 512, (c + 1) * 512)
                kps = ps.tile([128, 1024], f32, tag="qp", bufs=3, name="kps")
                for d in range(8):
                    xt = xkS3[:, d, s]
                    for n in range(2):
                        nc.tensor.matmul(kps[:, n * 512:(n + 1) * 512],
                                         lhsT=wkS3[:, d, n * 128:(n + 1) * 128],
                                         rhs=xt, start=(d == 0), stop=(d == 7))
                kraw = ev.tile([128, 1024], bf16, tag="qraw", bufs=2, name="kraw")
                for n in range(2):
                    nc.scalar.activation(out=kraw[:, n * 512:(n + 1) * 512],
                                         in_=kps[:, n * 512:(n + 1) * 512],
                                         func=IDENT, bias=bqkS[:, 2 + n:3 + n])
                rope_sbuf(kraw, kT, s)
                vps = ps.tile([128, 1024], f32, tag="up", bufs=1, name="vps")
                for i in range(4):
                    for d in range(9):
                        nc.tensor.matmul(vps[:, i * 256:(i + 1) * 256],
                                         lhsT=xvS3[:, d, s][:, i * 128:(i + 1) * 128],
                                         rhs=wvS3[:, d, :],
                                         start=(d == 0), stop=(d == 8))
                vraw = ev.tile([128, 1024], bf16, tag="eu", bufs=2, name="vraw")
                nc.scalar.activation(out=vraw, in_=vps, func=COPY)
                nc.vector.tensor_mul(
                    vg4[:, c * 4:(c + 1) * 4, :, 0:64],
                    vraw.rearrange("p (i h e) -> p i h e", h=4, e=64),
                    sig[c].rearrange("p (i h e) -> p i h e", h=4, e=64))

            # ---- Attention ----
            # heads in {2,3,0,1} order so oT[1] completes first and the
            # out-projection (n2=1-major) overlaps the last head's normalize.
            # One (h, hq) pass per 2-bank pvt accumulator; st triple-buffered
            # and PV matmuls lagging the scores by 3 tiles ACROSS pass
            # boundaries (one continuous pipeline), so the exp producers'
            # latency never creates a PE micro-gap (which would drop the
            # Tensor engine out of its top p-state).  Each pass's epilogue is
            # emitted when its last PV pops, and the reciprocal/broadcast
            # chain is split per hq half so only the final half-chain trails
            # the last PV.
            def epilogue(h, n, r0, hq, pvtH):
                for s2 in range(2):
                    qc = hq * 2 + s2
                    csl = slice(s2 * 512, s2 * 512 + 512)
                    rdst = rg[qc * 32:qc * 32 + 1, :]
                    if s2 == 0:
                        nc.vector.tensor_copy(out=rdst, in_=pvtH[64:65, csl])
                    else:
                        nc.scalar.activation(out=rdst, in_=pvtH[64:65, csl],
                                             func=COPY)
                for s2 in range(2):
                    qc = hq * 2 + s2
                    csl = slice(s2 * 512, s2 * 512 + 512)
                    dst = oT[n][r0:r0 + 64, qc * 512:(qc + 1) * 512]
                    if s2 == 0:
                        nc.scalar.activation(out=dst, in_=pvtH[0:64, csl],
                                             func=COPY)
                    else:
                        nc.vector.tensor_copy(out=dst, in_=pvtH[0:64, csl])
                if hq == 0:
                    return
                # whole-head chain: reciprocal -> DRAM broadcast -> scale
                nc.vector.reciprocal_approx_fast(out=rinv, in_=rg)
                drv = drm.tile([4, 512], f32, tag="drv", name="drv")
                nc.sync.dma_start(
                    out=drv,
                    in_=rinv.rearrange("(a b) f -> a b f", b=32)[:, 0, :])
                if h == 1:
                    nc.sync.dma_start(
                        out=ib[r0:r0 + 64, 0:1024],
                        in_=drv.flatten()[0:1024].partition_broadcast(64))
                    nc.scalar.dma_start(
                        out=ib[r0:r0 + 64, 1024:2048],
                        in_=drv.flatten()[1024:2048].partition_broadcast(64))
                    nc.vector.tensor_mul(oT[n][r0:r0 + 64, 0:1024],
                                         oT[n][r0:r0 + 64, 0:1024],
                                         ib[r0:r0 + 64, 0:1024])
                    nc.gpsimd.tensor_mul(oT[n][r0:r0 + 64, 1024:2048],
                                         oT[n][r0:r0 + 64, 1024:2048],
                                         ib[r0:r0 + 64, 1024:2048])
                else:
                    nc.sync.dma_start(out=ib[r0:r0 + 64, :],
                                      in_=drv.flatten()[:].partition_broadcast(64))
                    nc.gpsimd.tensor_mul(oT[n][r0:r0 + 64, :],
                                         oT[n][r0:r0 + 64, :], ib[r0:r0 + 64, :])

            def flush_one():
                pt, kt, fh, fn, fr0, fhq, fpvt = pending.pop(0)
                for s2 in range(2):
                    nc.tensor.matmul(
                        fpvt[0:65, s2 * 512:(s2 + 1) * 512],
                        lhsT=vg[:, kt * 260 + fh * 65:kt * 260 + fh * 65 + 65],
                        rhs=pt[:, s2 * 512:(s2 + 1) * 512],
                        start=(kt == 0), stop=(kt == 15))
                if kt == 15:
                    epilogue(fh, fn, fr0, fhq, fpvt)

            pending = []
            for h in (2, 3, 0, 1):
                n = h // 2
                r0 = (h % 2) * 64
                for hq in range(2):
                    # drain at the pass boundary: emits the previous pass's
                    # last PVs + epilogue before the next pass's scores, so
                    # the pvt slot is released well before its next first
                    # write and the epilogue ops get a head start.
                    while pending:
                        flush_one()
                    pvtH = ps.tile([128, 1024], f32, tag="up", bufs=1,
                                   name=f"pvt{h}_{hq}")
                    for kt in range(16):
                        col = kt * 4 + h
                        st = ps.tile([128, 1024], f32, tag="qp", bufs=3, name="st")
                        for s2 in range(2):
                            q0 = hq * 1024 + s2 * 512
                            nc.tensor.matmul(
                                st[:, s2 * 512:(s2 + 1) * 512],
                                lhsT=kT[n][r0:r0 + 64, kt * 128:(kt + 1) * 128],
                                rhs=qT[n][r0:r0 + 64, q0:q0 + 512],
                                start=True, stop=True)
                        if with_mask:
                            mt_ = mkpool.tile([128, 1024], f32, tag="mt", name="mt")
                            nc.sync.dma_start(
                                out=mt_,
                                in_=mk[kt * 128:(kt + 1) * 128,
                                       hq * 1024:(hq + 1) * 1024])
                            nc.vector.tensor_add(st, st, mt_)
                        pt = ptpool.tile([128, 1024], bf16, tag="pt", name="pt")
                        if use_dve(kt):
                            nc.vector.tensor_scalar(pt[:, :].bitcast(i16), st,
                                                    A16 * SCALE,
                                                    cbA[:, col:col + 1],
                                                    MULT, ADD)
                        else:
                            nc.scalar.activation(out=pt, in_=st, func=EXP,
                                                 scale=SCALE,
                                                 bias=cbS[:, col:col + 1])
                        pending.append((pt, kt, h, n, r0, hq, pvtH))
                        if len(pending) > 4:
                            flush_one()
            while pending:
                flush_one()

            # keep the PE p-state hot across the final normalize half-chain
            ogw = ps.tile([128, 1024], f32, tag="qp", bufs=3, name="ogwarm")
            for j in range(24):
                nc.tensor.matmul(ogw[:, 0:512], lhsT=woS[0][:, 0:128],
                                 rhs=woS[0][:, 0:512],
                                 start=(j == 0), stop=(j == 23))

            # ---- Out-projection (n2-major accumulation) ----
            for mt_i in range(8):
                og = [ps.tile([128, 1024], f32, tag="qp", bufs=3,
                              name=f"og{mt_i}_a"),
                      ps.tile([128, 1024], f32, tag="up", bufs=1,
                              name=f"og{mt_i}_b")]
                for n2 in (1, 0):
                    for qc in range(4):
                        nc.tensor.matmul(
                            og[qc // 2][:, (qc % 2) * 512:(qc % 2) * 512 + 512],
                            lhsT=woS[n2][:, mt_i * 128:(mt_i + 1) * 128],
                            rhs=oT[n2][:, qc * 512:(qc + 1) * 512],
                            start=(n2 == 1), stop=(n2 == 0))
                ot = otpool.tile([128, L], bf16, tag="ot", bufs=2, name="ot")
                nc.scalar.activation(out=ot[:, 0:1024], in_=og[0], func=COPY)
                nc.vector.tensor_copy(out=ot[:, 1024:2048], in_=og[1])
                ring = (nc.sync, nc.gpsimd, nc.scalar)[mt_i % 3]
                ring.dma_start(out=outT[mt_i * 128:(mt_i + 1) * 128, :], in_=ot)

    nc.finalize()
    return nc


def prep_inputs(query, key, value, attn_mask, action_ids, time_deltas,
                Wq, bq, Wk, bk, Wv, bv, Wu, bu, Wo, bo,
                action_emb, Wap, bap, td_emb, td_gate):
    """Host-side sharding: build the 8 per-core input maps."""
    query = np.asarray(query, np.float32)
    key = np.asarray(key, np.float32)
    value = np.asarray(value, np.float32)
    attn_mask = np.asarray(attn_mask)
    action_ids = np.asarray(action_ids)
    time_deltas = np.asarray(time_deltas)
    Wq, bq = np.asarray(Wq, np.float32), np.asarray(bq, np.float32)
    Wk, bk = np.asarray(Wk, np.float32), np.asarray(bk, np.float32)
    Wv, bv = np.asarray(Wv, np.float32), np.asarray(bv, np.float32)
    Wu, bu = np.asarray(Wu, np.float32), np.asarray(bu, np.float32)
    Wap, bap = np.asarray(Wap, np.float32), np.asarray(bap, np.float32)

    sig_gate = 1.0 / (1.0 + np.exp(-np.float64(td_gate)))
    with_mask = not bool(attn_mask.all())

    xq_b, xk_b, xv_b, cb_b, mk_b = [], [], [], [], []
    for b in range(B):
        ae = np.asarray(action_emb, np.float32)[action_ids[b]]      # [L, 16]
        xqa = np.zeros((9 * 128, L), BF16)
        xqa[:D] = query[b].T.astype(BF16)
        xqa[D:D + 16] = ae.T.astype(BF16)
        xqa[D + 16] = BF16(1.0)
        xq_b.append(xqa)
        xk_b.append(np.ascontiguousarray(key[b].T.astype(BF16)))    # [1024, L]
        xva = np.zeros((9 * 128, L), BF16)
        xva[:D] = value[b].T.astype(BF16)
        xva[D] = BF16(1.0)
        xv_b.append(xva)
        tdc = np.clip(time_deltas[b].astype(np.int64), 0, td_emb.shape[0] - 1)
        cb_b.append((sig_gate * np.asarray(td_emb, np.float32)[tdc]).astype(np.float32))
        if with_mask:
            m = np.where(attn_mask[b], np.float32(0.0), np.float32(-1e9))
            mk_b.append(np.ascontiguousarray(m.T))                  # [k, q]

    wu_a = np.zeros((9 * 128, D), np.float32)
    wu_a[:D] = Wu
    wu_a[D:D + 16] = Wap
    wu_a[D + 16] = bu + bap
    wv_a = np.zeros((9 * 128, D), np.float32)
    wv_a[:D] = Wv
    wv_a[D] = bv

    # RoPE tables in [dh, pos] orientation, duplicated for the 2-head packing.
    # Head dims are stored pair-interleaved (perm64) so the rotate_half
    # partner of partition p is p^1 (a 32-lane stream_shuffle pair swap); the
    # sin table carries the rotate_half sign.
    inv_freq = 1.0 / (10000.0 ** (np.arange(0, DH, 2, dtype=np.float64) / DH))
    pos = np.arange(L, dtype=np.float64)
    freqs = pos[None, :] * inv_freq[:, None]            # [32, L]
    cos_t = np.repeat(np.cos(freqs), 2, axis=0)[:DH]    # [64, L]
    sin_t = np.repeat(np.sin(freqs), 2, axis=0)[:DH]
    ss_t = sin_t.copy()
    ss_t[0:32] = -ss_t[0:32]
    perm64 = np.empty(DH, np.int64)
    perm64[0::2] = np.arange(32)
    perm64[1::2] = np.arange(32) + 32
    gperm = np.concatenate([h * DH + perm64 for h in range(4)])     # [256]
    cos_p, ss_p = cos_t[perm64], ss_t[perm64]
    cs_t = np.ascontiguousarray(np.concatenate([cos_p, cos_p], 0)).astype(BF16)
    sn_t = np.ascontiguousarray(np.concatenate([ss_p, ss_p], 0)).astype(BF16)

    in_maps = []
    for c in range(NCORES):
        b, hg = c // 4, c % 4
        csl = slice(hg * NG, (hg + 1) * NG)
        cbc = cb_b[b][:, hg * 4:(hg + 1) * 4]                       # [L, 4]
        cbc = cbc.reshape(16, 128, 4).transpose(1, 0, 2).reshape(128, 64)
        bq_g, bk_g = bq[csl][gperm], bk[csl][gperm]
        bqk_t = np.zeros((128, 4), np.float32)
        bqk_t[:, 0] = bq_g[0:128]
        bqk_t[:, 1] = bq_g[128:256]
        bqk_t[:, 2] = bk_g[0:128]
        bqk_t[:, 3] = bk_g[128:256]
        m = {
            "xq": xq_b[b], "xk": xk_b[b], "xv": xv_b[b],
            "wq": np.ascontiguousarray(Wq[:, csl][:, gperm]).astype(BF16),
            "wu": wu_a[:, csl].astype(BF16),
            "wk": np.ascontiguousarray(Wk[:, csl][:, gperm]).astype(BF16),
            "wv": wv_a[:, csl].astype(BF16),
            "wo": np.asarray(Wo, np.float32)[csl, :].astype(BF16),
            "cb": np.ascontiguousarray(cbc, np.float32),
            "bqk": bqk_t,
            "cs": cs_t, "sn": sn_t,
        }
        if with_mask:
            m["mk"] = mk_b[b]
        in_maps.append(m)
    return in_maps, with_mask


def gather_output(results, bo):
    """Sum head-group partials per batch, transpose, add bo."""
    out = np.empty((B, L, D), np.float32)
    for b in range(B):
        acc = results[b * 4]["outT"].astype(np.float32)
        for g in range(1, 4):
            acc = acc + results[b * 4 + g]["outT"].astype(np.float32)
        out[b] = acc.T + np.asarray(bo, np.float32)
    return out


def kernel(**inputs):
    from concourse.bass_utils import run_bass_kernel_spmd

    in_maps, with_mask = prep_inputs(**inputs)
    nc = build_bass(with_mask)
    res = run_bass_kernel_spmd(nc, in_maps, core_ids=list(range(NCORES)))
    return gather_output(res.results, inputs["bo"])


# revision 42
# speedup vs baseline: 1.1463x; 1.1463x over previous
"""Trainium2 Bass kernel for FlashMultiHeadAttention (B=2, L=2048, D=1024, H=16, Dh=64).

Sharding: 8 cores = 2 (batch) x 4 (head groups of 4 heads).
Per core (batch b, head group hg, 4 heads):
  - Q^T/K^T projections ([256, L], head dims on partitions, 8 k-tiles; bq/bk
    folded into the ACT-engine PSUM->SBUF evacuation as per-partition biases).
    RoPE applied by the DVE entirely in SBUF bf16 (4x DVE rate) on the ACT
    copies.  U (9 k-tiles, action-gate + biases via augmented rows) and V
    (9 k-tiles, ones-row bias) projected in natural [L, 256] layout; sigmoid
    via ACT tanh; gating mul fused into one DVE op per chunk.
  - Scores computed transposed (S^T[k, q]); exp producers alternate by key
    block between the ACT engine (exp with fused scale+bias+cast) and the DVE
    (Schraudolph int16 bitcast-to-bf16 approximation) so softmax never paces
    the PE.  P^T feeds PV directly; PV carries a denominator ones-column.
  - Normalization: fast-approx reciprocal + DRAM-broadcast of 1/r; the final
    per-head scale runs on GpSimd (SBUF only), off every critical engine.
  - Row-sliced output projection -> partial outT [1024, 2048] bf16, with the
    two 128-row halves of the contraction accumulated n2-major so the last
    head's normalize overlaps the first half of the out-projection.
Host sums the 4 head-group partials per batch and adds bo.

One PSUM pool with two 2-bank tags ("qp"/"up", bufs=2 each = 8 banks) is
shared by every phase so there are no pool-release barriers.  Input DMAs are
spread over four rings (SP: xq+xv, Pool: xk+wk/wv/wo, PE: wq/wu, ACT: small
tables + first xq chunk tail) so no single ring serializes the head.
"""

import sys

if "/opt/trn_rl_repo" not in sys.path:
    sys.path.insert(0, "/opt/trn_rl_repo")

import numpy as np
import ml_dtypes

BF16 = ml_dtypes.bfloat16

B = 2
L = 2048
D = 1024
H = 16
DH = 64
NG = 256          # head dims per group (4 heads)
NCORES = 8
SCALE = DH ** -0.5

# Schraudolph exp approximation constants (bf16 bit layout via int16).
A16 = 128.0 / float(np.log(2.0))
SIGMA = 0.0579
B0 = 128.0 * (127.0 - SIGMA)
# exp producer split: these key blocks' exp runs on the DVE (Schraudolph),
# the rest on the ACT engine, so the two producers run concurrently and
# neither paces the PE.
DVE_KT = (2, 5, 8, 10, 13, 15)


def build_bass(with_mask: bool):
    """Build the single-core SPMD Bass program (same program on all 8 cores)."""
    import concourse.mybir as mybir
    from concourse import bacc
    from concourse.tile import TileContext

    f32 = mybir.dt.float32
    bf16 = mybir.dt.bfloat16
    i16 = mybir.dt.int16
    EXP = mybir.ActivationFunctionType.Exp
    TANH = mybir.ActivationFunctionType.Tanh
    IDENT = mybir.ActivationFunctionType.Identity
    COPY = mybir.ActivationFunctionType.Copy
    MULT = mybir.AluOpType.mult
    ADD = mybir.AluOpType.add

    def use_dve(kt):
        return (not with_mask) and kt in DVE_KT

    nc = bacc.Bacc(None, target_bir_lowering=False)

    xq = nc.dram_tensor("xq", [9 * 128, L], bf16, kind="ExternalInput")
    xk = nc.dram_tensor("xk", [8 * 128, L], bf16, kind="ExternalInput")
    xv = nc.dram_tensor("xv", [9 * 128, L], bf16, kind="ExternalInput")
    wq = nc.dram_tensor("wq", [8 * 128, NG], bf16, kind="ExternalInput")
    wu = nc.dram_tensor("wu", [9 * 128, NG], bf16, kind="ExternalInput")
    wk = nc.dram_tensor("wk", [8 * 128, NG], bf16, kind="ExternalInput")
    wv = nc.dram_tensor("wv", [9 * 128, NG], bf16, kind="ExternalInput")
    wo = nc.dram_tensor("wo", [NG, D], bf16, kind="ExternalInput")
    cb = nc.dram_tensor("cb", [128, 64], f32, kind="ExternalInput")
    bqk = nc.dram_tensor("bqk", [128, 4], f32, kind="ExternalInput")
    cs = nc.dram_tensor("cs", [128, L], bf16, kind="ExternalInput")
    sn = nc.dram_tensor("sn", [128, L], bf16, kind="ExternalInput")
    mk = None
    if with_mask:
        mk = nc.dram_tensor("mk", [L, L], f32, kind="ExternalInput")
    outT = nc.dram_tensor("outT", [D, L], bf16, kind="ExternalOutput")

    with TileContext(nc) as tc:
        with tc.tile_pool(name="persist", bufs=1) as persist, \
             tc.tile_pool(name="xbuf", bufs=1) as xbuf, \
             tc.tile_pool(name="ps", bufs=1, space="PSUM") as ps, \
             tc.tile_pool(name="ev", bufs=2) as ev, \
             tc.tile_pool(name="ptp", bufs=6) as ptpool, \
             tc.tile_pool(name="otp", bufs=2) as otpool, \
             tc.tile_pool(name="drm", bufs=2, space="DRAM") as drm, \
             tc.tile_pool(name="mkp", bufs=4) as mkpool:
            qT = [persist.tile([128, L], bf16, name=f"qT{n}") for n in range(2)]
            kT = [persist.tile([128, L], bf16, name=f"kT{n}") for n in range(2)]
            vg = persist.tile([128, 16 * 260], bf16, name="vg")
            vg4 = vg.rearrange("p (t h e) -> p t h e", h=4, e=65)
            sig = [persist.tile([128, 1024], bf16, name=f"sig{c}") for c in range(4)]
            oT = [persist.tile([128, L], bf16, name=f"oT{n}") for n in range(2)]
            csS = persist.tile([128, L], bf16, name="csS")
            snS = persist.tile([128, L], bf16, name="snS")
            cbS = persist.tile([128, 64], f32, name="cbS")
            cbA = persist.tile([128, 64], f32, name="cbA")
            bqkS = persist.tile([128, 4], f32, name="bqkS")
            woS = [persist.tile([128, D], bf16, name=f"woS{n2}") for n2 in range(2)]
            wqS = persist.tile([128, 8 * NG], bf16, name="wqS")
            wuS = persist.tile([128, 9 * NG], bf16, name="wuS")
            wkS = persist.tile([128, 8 * NG], bf16, name="wkS")
            wvS = persist.tile([128, 9 * NG], bf16, name="wvS")
            ib = persist.tile([128, L], f32, name="ib")
            rg = persist.tile([128, 512], f32, name="rg")
            rinv = persist.tile([128, 512], f32, name="rinv")
            rinvb = [persist.tile([128, 512], bf16, name=f"rinvb{j}")
                     for j in range(2)]

            wqS3 = wqS.rearrange("p (c n) -> p c n", n=NG)
            wuS3 = wuS.rearrange("p (c n) -> p c n", n=NG)
            wkS3 = wkS.rearrange("p (c n) -> p c n", n=NG)
            wvS3 = wvS.rearrange("p (c n) -> p c n", n=NG)

            xqS = xbuf.tile([128, 9 * L], bf16, tag="xA", name="xqS")
            xqS3 = xqS.rearrange("p (c q) -> p c q", q=L)
            xkS = xbuf.tile([128, 8 * L], bf16, tag="xB", name="xkS")
            xkS3 = xkS.rearrange("p (c q) -> p c q", q=L)

            # ---- input DMAs spread over three rings ----
            # Pool ring: xq chunk 1 (the SP ring alone can't keep ahead of
            # the PE), then rope tables, xk, K/V weights, wo.
            for d in range(9):
                nc.gpsimd.dma_start(out=xqS3[:, d, 512:1024],
                                    in_=xq[d * 128:(d + 1) * 128, 512:1024])
            nc.gpsimd.dma_start(out=csS, in_=cs[:, :])
            nc.gpsimd.dma_start(out=snS, in_=sn[:, :])
            for d in range(8):
                nc.gpsimd.dma_start(out=xkS3[:, d, 0:512],
                                    in_=xk[d * 128:(d + 1) * 128, 0:512])
            nc.gpsimd.dma_start(out=wkS.rearrange("p (c n) -> p c n", n=NG),
                                in_=wk.rearrange("(c p) n -> p c n", p=128))
            nc.gpsimd.dma_start(out=wvS.rearrange("p (c n) -> p c n", n=NG),
                                in_=wv.rearrange("(c p) n -> p c n", p=128))
            for c in range(1, 4):
                s = slice(c * 512, (c + 1) * 512)
                for d in range(8):
                    nc.gpsimd.dma_start(out=xkS3[:, d, s],
                                        in_=xk[d * 128:(d + 1) * 128, s])
            for n2 in range(2):
                nc.gpsimd.dma_start(out=woS[n2], in_=wo[n2 * 128:(n2 + 1) * 128, :])
            # ACT ring: wq/wu ahead of the first matmuls, then small tables.
            nc.scalar.dma_start(out=wqS3, in_=wq.rearrange("(c p) n -> p c n", p=128))
            nc.scalar.dma_start(out=bqkS, in_=bqk[:, :])
            nc.scalar.dma_start(out=wuS3, in_=wu.rearrange("(c p) n -> p c n", p=128))
            nc.scalar.dma_start(out=cbS, in_=cb[:, :])
            # SP ring: xq chunks 0, 2, 3.
            for c in (0, 2, 3):
                s = slice(c * 512, (c + 1) * 512)
                for d in range(9):
                    nc.sync.dma_start(out=xqS3[:, d, s],
                                      in_=xq[d * 128:(d + 1) * 128, s])

            # device-side preamble computations
            nc.vector.tensor_scalar(cbA, cbS, A16, B0, MULT, ADD)
            nc.vector.memset(vg4[:, :, :, 64:65], 1.0)
            nc.gpsimd.memset(rg, 1.0)

            # head dims are stored pair-interleaved (partner of p is p^1), so
            # rotate_half is a swap of adjacent partitions within quadrants.
            SWAP_MASK = [i ^ 1 for i in range(32)]

            def rope_sbuf(raw, dest, s):
                """dest[:, s] = raw*cos + rotate_half(raw)*signed_sin (bf16 SBUF)."""
                for n in range(2):
                    src = raw[:, n * 512:(n + 1) * 512]
                    tcx = ev.tile([128, 512], bf16, tag="tc", bufs=2, name="tcx")
                    rot = ev.tile([128, 512], bf16, tag="tr", bufs=2, name="rot")
                    nc.vector.tensor_mul(tcx, src, csS[:, s])
                    nc.vector.stream_shuffle(rot, src, SWAP_MASK)
                    nc.vector.tensor_mul(rot, rot, snS[:, s])
                    nc.vector.tensor_add(dest[n][:, s], tcx, rot)

            # ---- QU phase ----
            for c in range(4):
                s = slice(c * 512, (c + 1) * 512)
                qps = ps.tile([128, 1024], f32, tag="qp", bufs=3, name="qps")
                for d in range(8):
                    xt = xqS3[:, d, s]
                    for n in range(2):
                        nc.tensor.matmul(qps[:, n * 512:(n + 1) * 512],
                                         lhsT=wqS3[:, d, n * 128:(n + 1) * 128],
                                         rhs=xt, start=(d == 0), stop=(d == 7))
                qraw = ev.tile([128, 1024], bf16, tag="qraw", bufs=2, name="qraw")
                for n in range(2):
                    nc.scalar.activation(out=qraw[:, n * 512:(n + 1) * 512],
                                         in_=qps[:, n * 512:(n + 1) * 512],
                                         func=IDENT, bias=bqkS[:, n:n + 1])
                rope_sbuf(qraw, qT, s)
                ups = ps.tile([128, 1024], f32, tag="up", bufs=1, name="ups")
                for i in range(4):
                    for d in range(9):
                        nc.tensor.matmul(ups[:, i * 256:(i + 1) * 256],
                                         lhsT=xqS3[:, d, s][:, i * 128:(i + 1) * 128],
                                         rhs=wuS3[:, d, :],
                                         start=(d == 0), stop=(d == 8))
                eu = ev.tile([128, 1024], bf16, tag="eu", bufs=2, name="eu")
                nc.scalar.activation(out=eu, in_=ups, func=TANH, scale=0.5)
                # sigmoid(u) = 0.5*tanh(0.5*u) + 0.5
                nc.vector.tensor_scalar(sig[c], eu, 0.5, 0.5, MULT, ADD)

            # ---- KV phase (xv reuses the xq SBUF bytes) ----
            xvS = xbuf.tile([128, 9 * L], bf16, tag="xA", name="xvS")
            xvS3 = xvS.rearrange("p (c q) -> p c q", q=L)
            for c in range(4):
                s = slice(c * 512, (c + 1) * 512)
                for d in range(9):
                    nc.sync.dma_start(out=xvS3[:, d, s],
                                      in_=xv[d * 128:(d + 1) * 128, s])
            for c in range(4):
                s = slice(c * 512, (c + 1) * 512)
                kps = ps.tile([128, 1024], f32, tag="qp", bufs=3, name="kps")
                for d in range(8):
                    xt = xkS3[:, d, s]
                    for n in range(2):
                        nc.tensor.matmul(kps[:, n * 512:(n + 1) * 512],
                                         lhsT=wkS3[:, d, n * 128:(n + 1) * 128],
                                         rhs=xt, start=(d == 0), stop=(d == 7))
                kraw = ev.tile([128, 1024], bf16, tag="qraw", bufs=2, name="kraw")
                for n in range(2):
                    nc.scalar.activation(out=kraw[:, n * 512:(n + 1) * 512],
                                         in_=kps[:, n * 512:(n + 1) * 512],
                                         func=IDENT, bias=bqkS[:, 2 + n:3 + n])
                rope_sbuf(kraw, kT, s)
                vps = ps.tile([128, 1024], f32, tag="up", bufs=1, name="vps")
                for i in range(4):
                    for d in range(9):
                        nc.tensor.matmul(vps[:, i * 256:(i + 1) * 256],
                                         lhsT=xvS3[:, d, s][:, i * 128:(i + 1) * 128],
                                         rhs=wvS3[:, d, :],
                                         start=(d == 0), stop=(d == 8))
                vraw = ev.tile([128, 1024], bf16, tag="eu", bufs=2, name="vraw")
                nc.scalar.activation(out=vraw, in_=vps, func=COPY)
                nc.vector.tensor_mul(
                    vg4[:, c * 4:(c + 1) * 4, :, 0:64],
                    vraw.rearrange("p (i h e) -> p i h e", h=4, e=64),
                    sig[c].rearrange("p (i h e) -> p i h e", h=4, e=64))

            # ---- Attention ----
            # heads in {2,3,0,1} order so oT[1] completes first and the
            # out-projection (n2=1-major) overlaps the last head's normalize.
            # One (h, hq) pass per 2-bank pvt accumulator; st triple-buffered
            # and PV matmuls lagging the scores by 3 tiles ACROSS pass
            # boundaries (one continuous pipeline), so the exp producers'
            # latency never creates a PE micro-gap (which would drop the
            # Tensor engine out of its top p-state).  Each pass's epilogue is
            # emitted when its last PV pops, and the reciprocal/broadcast
            # chain is split per hq half so only the final half-chain trails
            # the last PV.
            def epilogue(h, n, r0, hq, pvtH):
                for s2 in range(2):
                    qc = hq * 2 + s2
                    csl = slice(s2 * 512, s2 * 512 + 512)
                    rdst = rg[qc * 32:qc * 32 + 1, :]
                    if s2 == 0:
                        nc.vector.tensor_copy(out=rdst, in_=pvtH[64:65, csl])
                    else:
                        nc.scalar.activation(out=rdst, in_=pvtH[64:65, csl],
                                             func=COPY)
                for s2 in range(2):
                    qc = hq * 2 + s2
                    csl = slice(s2 * 512, s2 * 512 + 512)
                    dst = oT[n][r0:r0 + 64, qc * 512:(qc + 1) * 512]
                    if s2 == 0:
                        nc.scalar.activation(out=dst, in_=pvtH[0:64, csl],
                                             func=COPY)
                    else:
                        nc.vector.tensor_copy(out=dst, in_=pvtH[0:64, csl])
                if hq == 0:
                    return
                # whole-head chain: reciprocal -> DRAM broadcast -> scale
                nc.vector.reciprocal_approx_fast(out=rinv, in_=rg)
                drv = drm.tile([4, 512], f32, tag="drv", name="drv")
                nc.sync.dma_start(
                    out=drv,
                    in_=rinv.rearrange("(a b) f -> a b f", b=32)[:, 0, :])
                nc.sync.dma_start(out=ib[r0:r0 + 64, :],
                                  in_=drv.flatten()[:].partition_broadcast(64))
                eng = nc.vector if h == 1 else nc.gpsimd
                eng.tensor_mul(oT[n][r0:r0 + 64, :],
                               oT[n][r0:r0 + 64, :], ib[r0:r0 + 64, :])

            def flush_one():
                pt, kt, fh, fn, fr0, fhq, fpvt = pending.pop(0)
                for s2 in range(2):
                    nc.tensor.matmul(
                        fpvt[0:65, s2 * 512:(s2 + 1) * 512],
                        lhsT=vg[:, kt * 260 + fh * 65:kt * 260 + fh * 65 + 65],
                        rhs=pt[:, s2 * 512:(s2 + 1) * 512],
                        start=(kt == 0), stop=(kt == 15))
                if kt == 15:
                    epilogue(fh, fn, fr0, fhq, fpvt)

            pending = []
            for h in (2, 3, 0, 1):
                n = h // 2
                r0 = (h % 2) * 64
                for hq in range(2):
                    pvtH = ps.tile([128, 1024], f32, tag="up", bufs=1,
                                   name=f"pvt{h}_{hq}")
                    for kt in range(16):
                        col = kt * 4 + h
                        st = ps.tile([128, 1024], f32, tag="qp", bufs=3, name="st")
                        for s2 in range(2):
                            q0 = hq * 1024 + s2 * 512
                            nc.tensor.matmul(
                                st[:, s2 * 512:(s2 + 1) * 512],
                                lhsT=kT[n][r0:r0 + 64, kt * 128:(kt + 1) * 128],
                                rhs=qT[n][r0:r0 + 64, q0:q0 + 512],
                                start=True, stop=True)
                        if with_mask:
                            mt_ = mkpool.tile([128, 1024], f32, tag="mt", name="mt")
                            nc.sync.dma_start(
                                out=mt_,
                                in_=mk[kt * 128:(kt + 1) * 128,
                                       hq * 1024:(hq + 1) * 1024])
                            nc.vector.tensor_add(st, st, mt_)
                        pt = ptpool.tile([128, 1024], bf16, tag="pt", name="pt")
                        if use_dve(kt):
                            nc.vector.tensor_scalar(pt[:, :].bitcast(i16), st,
                                                    A16 * SCALE,
                                                    cbA[:, col:col + 1],
                                                    MULT, ADD)
                        else:
                            nc.scalar.activation(out=pt, in_=st, func=EXP,
                                                 scale=SCALE,
                                                 bias=cbS[:, col:col + 1])
                        pending.append((pt, kt, h, n, r0, hq, pvtH))
                        if len(pending) > 3:
                            flush_one()
            while pending:
                flush_one()

            # keep the PE p-state hot across the final normalize half-chain
            ogw = ps.tile([128, 1024], f32, tag="qp", bufs=3, name="ogwarm")
            for j in range(16):
                nc.tensor.matmul(ogw[:, 0:512], lhsT=woS[0][:, 0:128],
                                 rhs=woS[0][:, 0:512],
                                 start=(j == 0), stop=(j == 15))

            # ---- Out-projection (n2-major accumulation) ----
            for mt_i in range(8):
                og = [ps.tile([128, 1024], f32, tag="qp", bufs=3,
                              name=f"og{mt_i}_a"),
                      ps.tile([128, 1024], f32, tag="up", bufs=1,
                              name=f"og{mt_i}_b")]
                for n2 in (1, 0):
                    for qc in range(4):
                        nc.tensor.matmul(
                            og[qc // 2][:, (qc % 2) * 512:(qc % 2) * 512 + 512],
                            lhsT=woS[n2][:, mt_i * 128:(mt_i + 1) * 128],
                            rhs=oT[n2][:, qc * 512:(qc + 1) * 512],
                            start=(n2 == 1), stop=(n2 == 0))
                ot = otpool.tile([128, L], bf16, tag="ot", bufs=2, name="ot")
                nc.scalar.activation(out=ot[:, 0:1024], in_=og[0], func=COPY)
                nc.vector.tensor_copy(out=ot[:, 1024:2048], in_=og[1])
                ring = (nc.sync, nc.gpsimd, nc.scalar)[mt_i % 3]
                ring.dma_start(out=outT[mt_i * 128:(mt_i + 1) * 128, :], in_=ot)

    nc.finalize()
    return nc


def prep_inputs(query, key, value, attn_mask, action_ids, time_deltas,
                Wq, bq, Wk, bk, Wv, bv, Wu, bu, Wo, bo,
                action_emb, Wap, bap, td_emb, td_gate):
    """Host-side sharding: build the 8 per-core input maps."""
    query = np.asarray(query, np.float32)
    key = np.asarray(key, np.float32)
    value = np.asarray(value, np.float32)
    attn_mask = np.asarray(attn_mask)
    action_ids = np.asarray(action_ids)
    time_deltas = np.asarray(time_deltas)
    Wq, bq = np.asarray(Wq, np.float32), np.asarray(bq, np.float32)
    Wk, bk = np.asarray(Wk, np.float32), np.asarray(bk, np.float32)
    Wv, bv = np.asarray(Wv, np.float32), np.asarray(bv, np.float32)
    Wu, bu = np.asarray(Wu, np.float32), np.asarray(bu, np.float32)
    Wap, bap = np.asarray(Wap, np.float32), np.asarray(bap, np.float32)

    sig_gate = 1.0 / (1.0 + np.exp(-np.float64(td_gate)))
    with_mask = not bool(attn_mask.all())

    xq_b, xk_b, xv_b, cb_b, mk_b = [], [], [], [], []
    for b in range(B):
        ae = np.asarray(action_emb, np.float32)[action_ids[b]]      # [L, 16]
        xqa = np.zeros((9 * 128, L), BF16)
        xqa[:D] = query[b].T.astype(BF16)
        xqa[D:D + 16] = ae.T.astype(BF16)
        xqa[D + 16] = BF16(1.0)
        xq_b.append(xqa)
        xk_b.append(np.ascontiguousarray(key[b].T.astype(BF16)))    # [1024, L]
        xva = np.zeros((9 * 128, L), BF16)
        xva[:D] = value[b].T.astype(BF16)
        xva[D] = BF16(1.0)
        xv_b.append(xva)
        tdc = np.clip(time_deltas[b].astype(np.int64), 0, td_emb.shape[0] - 1)
        cb_b.append((sig_gate * np.asarray(td_emb, np.float32)[tdc]).astype(np.float32))
        if with_mask:
            m = np.where(attn_mask[b], np.float32(0.0), np.float32(-1e9))
            mk_b.append(np.ascontiguousarray(m.T))                  # [k, q]

    wu_a = np.zeros((9 * 128, D), np.float32)
    wu_a[:D] = Wu
    wu_a[D:D + 16] = Wap
    wu_a[D + 16] = bu + bap
    wv_a = np.zeros((9 * 128, D), np.float32)
    wv_a[:D] = Wv
    wv_a[D] = bv

    # RoPE tables in [dh, pos] orientation, duplicated for the 2-head packing.
    # Head dims are stored pair-interleaved (perm64) so the rotate_half
    # partner of partition p is p^1 (a 32-lane stream_shuffle pair swap); the
    # sin table carries the rotate_half sign.
    inv_freq = 1.0 / (10000.0 ** (np.arange(0, DH, 2, dtype=np.float64) / DH))
    pos = np.arange(L, dtype=np.float64)
    freqs = pos[None, :] * inv_freq[:, None]            # [32, L]
    cos_t = np.repeat(np.cos(freqs), 2, axis=0)[:DH]    # [64, L]
    sin_t = np.repeat(np.sin(freqs), 2, axis=0)[:DH]
    ss_t = sin_t.copy()
    ss_t[0:32] = -ss_t[0:32]
    perm64 = np.empty(DH, np.int64)
    perm64[0::2] = np.arange(32)
    perm64[1::2] = np.arange(32) + 32
    gperm = np.concatenate([h * DH + perm64 for h in range(4)])     # [256]
    cos_p, ss_p = cos_t[perm64], ss_t[perm64]
    cs_t = np.ascontiguousarray(np.concatenate([cos_p, cos_p], 0)).astype(BF16)
    sn_t = np.ascontiguousarray(np.concatenate([ss_p, ss_p], 0)).astype(BF16)

    in_maps = []
    for c in range(NCORES):
        b, hg = c // 4, c % 4
        csl = slice(hg * NG, (hg + 1) * NG)
        cbc = cb_b[b][:, hg * 4:(hg + 1) * 4]                       # [L, 4]
        cbc = cbc.reshape(16, 128, 4).transpose(1, 0, 2).reshape(128, 64)
        bq_g, bk_g = bq[csl][gperm], bk[csl][gperm]
        bqk_t = np.zeros((128, 4), np.float32)
        bqk_t[:, 0] = bq_g[0:128]
        bqk_t[:, 1] = bq_g[128:256]
        bqk_t[:, 2] = bk_g[0:128]
        bqk_t[:, 3] = bk_g[128:256]
        m = {
            "xq": xq_b[b], "xk": xk_b[b], "xv": xv_b[b],
            "wq": np.ascontiguousarray(Wq[:, csl][:, gperm]).astype(BF16),
            "wu": wu_a[:, csl].astype(BF16),
            "wk": np.ascontiguousarray(Wk[:, csl][:, gperm]).astype(BF16),
            "wv": wv_a[:, csl].astype(BF16),
            "wo": np.asarray(Wo, np.float32)[csl, :].astype(BF16),
            "cb": np.ascontiguousarray(cbc, np.float32),
            "bqk": bqk_t,
            "cs": cs_t, "sn": sn_t,
        }
        if with_mask:
            m["mk"] = mk_b[b]
        in_maps.append(m)
    return in_maps, with_mask


def gather_output(results, bo):
    """Sum head-group partials per batch, transpose, add bo."""
    out = np.empty((B, L, D), np.float32)
    for b in range(B):
        acc = results[b * 4]["outT"].astype(np.float32)
        for g in range(1, 4):
            acc = acc + results[b * 4 + g]["outT"].astype(np.float32)
        out[b] = acc.T + np.asarray(bo, np.float32)
    return out


def kernel(**inputs):
    from concourse.bass_utils import run_bass_kernel_spmd

    in_maps, with_mask = prep_inputs(**inputs)
    nc = build_bass(with_mask)
    res = run_bass_kernel_spmd(nc, in_maps, core_ids=list(range(NCORES)))
    return gather_output(res.results, inputs["bo"])
